# revision 31
# baseline (speedup 1.0000x reference)
"""Multi-Head Latent Attention (MLA) Bass kernel for 8 trn2 NeuronCores.

Sharding: core c handles batch b=c//4 and head group hg=c%4 (4 of 16 heads).
Host folds the q-path compression (W_dq @ W_uq_hg, W_dq @ W_qr_hg) into one
[2048, 768] matrix per core so q projects straight from x — no c_q compute.
Everything on-device runs in bf16 (fp32 PSUM accumulation), which lets all
intermediates (kcT/krT/vS) stay SBUF-resident across the whole kernel: the
only DRAM traffic is x in, weights in, out rows out.

Per t-block g (512 tokens): project q/kv for the block, then run causal
attention for q-block g over key chunks 0..4g+3 (all SBUF-resident), then the
W_o partial for those rows. The softmax row-sum reciprocal is broadcast
across partitions with a K=1 outer-product matmul instead of a DMA roundtrip.

Host sums the 4 partial outputs per batch.
"""

import numpy as np

T = 2048
C = 2048
QC = 1536
KV = 512
NH = 16
DH = 128
R = 64
TB = 512           # T block / q-group width
NTB = T // TB      # 4
SCALE = 1.0 / float(np.sqrt(DH + R))
ROPE_BASE = 10000.0

_CACHE = {}


def _build_nc(repeat=1):
    import concourse.bacc as bacc
    import concourse.mybir as mybir
    import concourse.tile as tile

    DT = mybir.dt.bfloat16
    F32 = mybir.dt.float32
    F32R = mybir.dt.float32r

    nc = bacc.Bacc("TRN2", target_bir_lowering=False, debug=False)

    xT = nc.dram_tensor("xT", [C, T], DT, kind="ExternalInput")
    wq = nc.dram_tensor("wq", [C, 768], DT, kind="ExternalInput")
    wdkv = nc.dram_tensor("wdkv", [C, KV], DT, kind="ExternalInput")
    wuk = nc.dram_tensor("wuk", [KV, 512], DT, kind="ExternalInput")
    wuv = nc.dram_tensor("wuv", [KV, 512], DT, kind="ExternalInput")
    wkr = nc.dram_tensor("wkr", [KV, 256], DT, kind="ExternalInput")
    wo = nc.dram_tensor("wo", [512, C], DT, kind="ExternalInput")
    cosd = nc.dram_tensor("cosd", [128, T], F32, kind="ExternalInput")
    sind = nc.dram_tensor("sind", [128, T], F32, kind="ExternalInput")
    maskd = nc.dram_tensor("maskd", [TB, TB], DT, kind="ExternalInput")
    onesd = nc.dram_tensor("onesd", [128, 1], DT, kind="ExternalInput")
    rotmd = nc.dram_tensor("rotmd", [128, 128], DT, kind="ExternalInput")
    out = nc.dram_tensor("out", [T, C], DT, kind="ExternalOutput")

    with tile.TileContext(nc) as tc:
        for _rep in range(repeat):
            _emit_body(nc, tc, mybir,
                       xT, wq, wdkv, wuk, wuv, wkr, wo,
                       cosd, sind, maskd, onesd, rotmd, out)

    nc.compile()
    return nc


def _emit_body(nc, tc, mybir,
               xT, wq, wdkv, wuk, wuv, wkr, wo,
               cosd, sind, maskd, onesd, rotmd, out):
    DT = mybir.dt.bfloat16
    F32 = mybir.dt.float32
    F32R = mybir.dt.float32r
    AF = mybir.ActivationFunctionType

    with (
        tc.tile_pool(name="sb", bufs=1) as sp,
        tc.tile_pool(name="ps", bufs=1, space="PSUM") as pp,
    ):
        # resident weights + tables. wq streams per-k-chunk on the sync queue
        # so block 0's first matmuls start as soon as chunk 0 lands; the x
        # block streams in parallel on the scalar queue.
        wq_sb = [sp.tile([128, 768], DT, name=f"wq_sb{k}") for k in range(16)]
        for k in range(16):
            nc.sync.dma_start(wq_sb[k][:], wq[128 * k:128 * (k + 1), :])
        wdkv_sb = sp.tile([128, 16, KV], DT, name="wdkv_sb")
        wuk_sb = sp.tile([128, 4, 512], DT, name="wuk_sb")
        wuv_sb = sp.tile([128, 4, 512], DT, name="wuv_sb")
        wkr_sb = sp.tile([128, 4, 256], DT, name="wkr_sb")
        wo_sb = sp.tile([128, 4, C], DT, name="wo_sb")
        mask_sb = sp.tile([128, 4, TB], DT, name="mask_sb")
        ones_sb = sp.tile([128, 1], DT, name="ones_sb")
        rotm_sb = sp.tile([128, 128], DT, name="rotm_sb")

        def load_late_weights():
            nc.scalar.dma_start(wdkv_sb[:], wdkv.rearrange("(k p) n -> p k n", p=128))
            nc.sync.dma_start(wuk_sb[:], wuk.rearrange("(k p) n -> p k n", p=128))
            nc.scalar.dma_start(wuv_sb[:], wuv.rearrange("(k p) n -> p k n", p=128))
            nc.sync.dma_start(wkr_sb[:], wkr.rearrange("(k p) n -> p k n", p=128))
            nc.sync.dma_start(mask_sb[:], maskd.rearrange("(j p) n -> p j n", p=128))
            nc.sync.dma_start(ones_sb[:], onesd[:])
            nc.sync.dma_start(rotm_sb[:], rotmd[:])
            nc.scalar.dma_start(wo_sb[:], wo.rearrange("(h p) n -> p h n", p=128))

        # prewarm the Act function table so the first real exp/copy isn't
        # serialized behind LoadActFuncSet (its input is never consumed)
        dum = sp.tile([1, 16], F32, name="dum")
        nc.scalar.activation(dum[:], dum[:], AF.Exp)

        # resident intermediates (whole-T K/V for this head group)
        kcT = sp.tile([128, 4, T], DT, name="kcT")
        krT = sp.tile([128, 2, T], DT, name="krT")
        vS = sp.tile([128, 16, 512], DT, name="vS")

        def rope_begin(ps_t, cos_sb):
            # ps_t [128, TB]: rows [64 head 2p | 64 head 2p+1] rope dims.
            # Stage raw rope input to SBUF (Act) and take the cos term (DVE);
            # both read PSUM so the bank frees after these two ops.
            st = sp.tile([128, TB], DT, name="rst", tag="rst", bufs=3)
            nc.scalar.copy(st[:], ps_t[:])
            t1 = sp.tile([128, TB], F32, name="rp1", tag="rp1", bufs=3)
            nc.vector.tensor_mul(t1[:], ps_t[:], cos_sb[:])
            return st, t1

        def rope_finish(rb, dst, sin_sb):
            # rotate-half as a signed-permutation matmul on PE; emitted a
            # matmul-group later so the staging copy is done by the time the
            # PE reaches it (PE executes its queue in program order).
            st, t1 = rb
            ps2 = pp.tile([128, TB], F32, name="ps_r", tag="ps", bufs=3)
            nc.tensor.matmul(ps2[:], rotm_sb[:], st[:], start=True, stop=True)
            sh = sp.tile([128, TB], F32, name="rp2", tag="rp2", bufs=2)
            nc.vector.tensor_mul(sh[:], ps2[:], sin_sb[:])
            nc.vector.tensor_add(dst, t1[:], sh[:])

        for gp in range(2):
            pair = (2 * gp, 2 * gp + 1)
            # stage x/cos/sin for both blocks of the pair; per-k tiles keep
            # DMA-arrival dependencies exact.
            xb, cosv, sinv = {}, {}, {}
            for g in pair:
                tc0 = TB * g
                xb[g] = [sp.tile([128, TB], DT, name=f"xblk{g % 2}_{k}",
                                 tag=f"xblk{g % 2}_{k}", bufs=1)
                         for k in range(16)]
                for k in range(16):
                    eng = nc.scalar if (gp == 0 or k % 2 == 0) else nc.sync
                    eng.dma_start(xb[g][k][:],
                                  xT[128 * k:128 * (k + 1), tc0:tc0 + TB])
                cosv[g] = sp.tile([128, TB], F32, name="cos_sb",
                                  tag=f"cos{g % 2}", bufs=1)
                nc.scalar.dma_start(cosv[g][:], cosd[:, tc0:tc0 + TB])
                sinv[g] = sp.tile([128, TB], F32, name="sin_sb",
                                  tag=f"sin{g % 2}", bufs=1)
                nc.scalar.dma_start(sinv[g][:], sind[:, tc0:tc0 + TB])
            if gp == 0:
                load_late_weights()

            def ps_pair():
                # one PSUM accumulator per block of the pair, on separate
                # tags so their WAR chains stay independent
                return {pair[0]: pp.tile([128, TB], F32, name="ps",
                                         tag="ps", bufs=3),
                        pair[1]: pp.tile([128, TB], F32, name="ps_s",
                                         tag="ps_qk", bufs=3)}

            # ---- projections, k-inner over the pair: each weight slice is
            # stationary for 2 consecutive matmuls ----
            qc_t = {g: sp.tile([128, 4, TB], DT, name="qc_t",
                               tag=f"qc_t{g % 2}", bufs=1) for g in pair}
            qr_t = {g: sp.tile([128, 2, TB], DT, name="qr_t",
                               tag=f"qr_t{g % 2}", bufs=1) for g in pair}
            qr_rb = {}
            for m in range(6):
                pst = ps_pair()
                for k in range(16):
                    for g in pair:
                        nc.tensor.matmul(pst[g][:],
                                         wq_sb[k][:, 128 * m:128 * (m + 1)],
                                         xb[g][k][:],
                                         start=(k == 0), stop=(k == 15))
                for g in pair:
                    if m < 4:
                        nc.vector.tensor_copy(qc_t[g][:, m, :], pst[g][:])
                    else:
                        qr_rb[(g, m - 4)] = rope_begin(pst[g], cosv[g])
            ckv = {g: sp.tile([128, 4, TB], DT, name="ckv",
                              tag=f"ckv{g % 2}", bufs=1) for g in pair}
            for m in range(4):
                pst = ps_pair()
                for k in range(16):
                    for g in pair:
                        nc.tensor.matmul(pst[g][:],
                                         wdkv_sb[:, k, 128 * m:128 * (m + 1)],
                                         xb[g][k][:],
                                         start=(k == 0), stop=(k == 15))
                for g in pair:
                    nc.vector.tensor_copy(ckv[g][:, m, :], pst[g][:])
                if m < 2:
                    # both blocks' rot matmuls back-to-back: they share the
                    # stationary rotation matrix
                    for g in pair:
                        rope_finish(qr_rb[(g, m)], qr_t[g][:, m, :], sinv[g])
            for h in range(4):
                pst = ps_pair()
                for k in range(4):
                    for g in pair:
                        nc.tensor.matmul(pst[g][:],
                                         wuk_sb[:, k, 128 * h:128 * (h + 1)],
                                         ckv[g][:, k, :],
                                         start=(k == 0), stop=(k == 3))
                for g in pair:
                    nc.vector.tensor_copy(kcT[:, h, TB * g:TB * (g + 1)],
                                          pst[g][:])
            kr_rb = {}
            for p in range(2):
                pst = ps_pair()
                for k in range(4):
                    for g in pair:
                        nc.tensor.matmul(pst[g][:],
                                         wkr_sb[:, k, 128 * p:128 * (p + 1)],
                                         ckv[g][:, k, :],
                                         start=(k == 0), stop=(k == 3))
                for g in pair:
                    kr_rb[(g, p)] = rope_begin(pst[g], cosv[g])
            for tkc in range(4):
                pst = ps_pair()
                for k in range(4):
                    for g in pair:
                        nc.tensor.matmul(pst[g][:],
                                         ckv[g][:, k, 128 * tkc:128 * (tkc + 1)],
                                         wuv_sb[:, k, :],
                                         start=(k == 0), stop=(k == 3))
                for g in pair:
                    nc.scalar.copy(vS[:, 4 * g + tkc, :], pst[g][:])
                if 1 <= tkc <= 2:
                    for g in pair:
                        rope_finish(kr_rb[(g, tkc - 1)],
                                    krT[:, tkc - 1, TB * g:TB * (g + 1)],
                                    sinv[g])

            # ---- attention + W_o per block of the pair ----
            for g in pair:
                tc0 = TB * g
                nch = 4 * (g + 1)
                avn = sp.tile([128, 4, TB], DT, name="avn", tag="avn", bufs=2)
                for h in range(4):
                    hp, r0 = h // 2, 64 * (h % 2)
                    ps_av = pp.tile([128, TB], F32, name="ps_av", tag="ps_av",
                                    bufs=1)
                    ps_sum = pp.tile([1, TB], F32, name="ps_sum", tag="ps_sum",
                                     bufs=1)

                    def qstart(c):
                        # columns tq < tk are fully masked; chunk 0 stays full
                        # (PSUM base write must cover all columns)
                        if c <= 4 * g or c == 0:
                            return 0
                        return 128 * (c - 4 * g)

                    def qk(c):
                        s = qstart(c)
                        ps_s = pp.tile([128, TB], F32, name="ps_s",
                                       tag="ps_qk", bufs=3)
                        nc.tensor.matmul(ps_s[:, s:],
                                         kcT[:, h, 128 * c:128 * (c + 1)],
                                         qc_t[g][:, h, s:],
                                         start=True, stop=False)
                        nc.tensor.matmul(ps_s[:, s:],
                                         krT[r0:r0 + 64, hp, 128 * c:128 * (c + 1)],
                                         qr_t[g][r0:r0 + 64, hp, s:],
                                         start=False, stop=True)
                        return ps_s

                    qkq = [qk(c) for c in range(min(3, nch))]
                    for c in range(nch):
                        cur = qkq.pop(0)
                        if c + 3 < nch:
                            qkq.append(qk(c + 3))
                        s = qstart(c)
                        ex = sp.tile([128, TB], DT, name="ex", tag="ex", bufs=4)
                        if c >= 4 * g:
                            er = sp.tile([128, TB], DT, name="er", tag="er",
                                         bufs=2)
                            nc.scalar.activation(er[:, s:], cur[:, s:], AF.Exp,
                                                 scale=SCALE)
                            nc.vector.tensor_mul(ex[:, s:], er[:, s:],
                                                 mask_sb[:, c - 4 * g, s:])
                        else:
                            nc.scalar.activation(ex[:, s:], cur[:, s:], AF.Exp,
                                                 scale=SCALE)
                        nc.tensor.matmul(ps_av[:, s:],
                                         vS[:, c, 128 * h:128 * (h + 1)],
                                         ex[:, s:], start=(c == 0),
                                         stop=(c == nch - 1))
                        nc.tensor.matmul(ps_sum[:, s:], ones_sb[:], ex[:, s:],
                                         start=(c == 0), stop=(c == nch - 1))
                    recip = sp.tile([1, TB], F32, name="recip", tag="recip",
                                    bufs=2)
                    nc.vector.reciprocal(recip[:], ps_sum[:])
                    bc = sp.tile([128, TB], F32, name="bc", tag="bc", bufs=1)
                    nc.gpsimd.partition_broadcast(bc[:], recip[:])
                    nc.vector.tensor_mul(avn[:, h, :], ps_av[:], bc[:])

                for tqc in range(4):
                    # h-outer/n-inner: 4 consecutive matmuls share the
                    # stationary avn chunk
                    pso = [pp.tile([128, 512], F32, name="ps_o", tag="ps_qk",
                                   bufs=3) for _ in range(2)]
                    pso += [pp.tile([128, 512], F32, name="ps", tag="ps",
                                    bufs=3) for _ in range(2)]
                    for h in range(4):
                        for n in range(4):
                            nc.tensor.matmul(
                                pso[n][:], avn[:, h, 128 * tqc:128 * (tqc + 1)],
                                wo_sb[:, h, 512 * n:512 * (n + 1)],
                                start=(h == 0), stop=(h == 3))
                    for n in range(4):
                        ost = sp.tile([128, 512], DT, name="ost", tag="ost",
                                      bufs=4)
                        if g == NTB - 1 or (tqc + n) % 2 == 0:
                            nc.vector.tensor_copy(ost[:], pso[n][:])
                        else:
                            nc.scalar.copy(ost[:], pso[n][:])
                        oeng = nc.sync if (tqc + n) % 2 == 0 else nc.scalar
                        oeng.dma_start(
                            out[tc0 + 128 * tqc:tc0 + 128 * (tqc + 1),
                                512 * n:512 * (n + 1)], ost[:])


def _rope_tables():
    inv = 1.0 / (ROPE_BASE ** (np.arange(0, R, 2, dtype=np.float32) / R))
    freqs = np.arange(T, dtype=np.float32)[:, None] * inv[None, :]       # [T, 32]
    emb = np.concatenate([freqs, freqs], axis=-1)                         # [T, 64]
    cosT = np.ascontiguousarray(np.cos(emb).T.astype(np.float32))         # [64, T]
    sinT = np.ascontiguousarray(np.sin(emb).T.astype(np.float32))
    cosd = np.concatenate([cosT, cosT], axis=0)                           # [128, T]
    sind = np.concatenate([sinT, sinT], axis=0)                           # [128, T]
    return cosd, sind


def _build_inmaps(x, W_dq, W_uq, W_qr, W_dkv, W_uk, W_uv, W_kr, W_o):
    import ml_dtypes
    BF16 = ml_dtypes.bfloat16

    cosd, sind = _rope_tables()
    maskv = (np.arange(TB)[:, None] <= np.arange(TB)[None, :]).astype(BF16)
    onesv = np.ones((128, 1), dtype=BF16)
    # rot(x)[i] = -x[i+32] (i<32), +x[i-32] (32<=i<64), per 64-row head block
    P64 = np.zeros((64, 64), dtype=np.float32)
    for i in range(32):
        P64[i, i + 32] = -1.0
        P64[i + 32, i] = 1.0
    Pm = np.zeros((128, 128), dtype=np.float32)
    Pm[0:64, 0:64] = P64
    Pm[64:128, 64:128] = P64
    rotmv = np.ascontiguousarray(Pm.T).astype(BF16)   # lhsT for out = Pm @ x

    # fold the q compression once for all heads, slice per core
    Wq_c = (W_dq @ W_uq).astype(np.float32)   # [2048, 2048]
    Wq_r = (W_dq @ W_qr).astype(np.float32)   # [2048, 1024]

    in_maps = []
    for core in range(8):
        b, hg = core // 4, core % 4
        wq_np = np.concatenate(
            [Wq_c[:, 512 * hg:512 * (hg + 1)], Wq_r[:, 256 * hg:256 * (hg + 1)]],
            axis=1).astype(BF16)
        in_maps.append({
            "xT": np.ascontiguousarray(x[b].T).astype(BF16),
            "wq": wq_np,
            "wdkv": W_dkv.astype(BF16),
            "wuk": np.ascontiguousarray(W_uk[:, 512 * hg:512 * (hg + 1)]).astype(BF16),
            "wuv": np.ascontiguousarray(W_uv[:, 512 * hg:512 * (hg + 1)]).astype(BF16),
            "wkr": np.ascontiguousarray(W_kr[:, 256 * hg:256 * (hg + 1)]).astype(BF16),
            "wo": np.ascontiguousarray(W_o[512 * hg:512 * (hg + 1), :]).astype(BF16),
            "cosd": cosd,
            "sind": sind,
            "maskd": maskv,
            "onesd": onesv,
            "rotmd": rotmv,
        })
    return in_maps


def kernel(**inputs):
    from concourse.bass_utils import run_bass_kernel_spmd

    x = np.asarray(inputs["x"], dtype=np.float32)
    W_dq = np.asarray(inputs["W_dq"], dtype=np.float32)
    W_uq = np.asarray(inputs["W_uq"], dtype=np.float32)
    W_qr = np.asarray(inputs["W_qr"], dtype=np.float32)
    W_dkv = np.asarray(inputs["W_dkv"], dtype=np.float32)
    W_uk = np.asarray(inputs["W_uk"], dtype=np.float32)
    W_uv = np.asarray(inputs["W_uv"], dtype=np.float32)
    W_kr = np.asarray(inputs["W_kr"], dtype=np.float32)
    W_o = np.asarray(inputs["W_o"], dtype=np.float32)

    if "nc" not in _CACHE:
        _CACHE["nc"] = _build_nc()
    nc = _CACHE["nc"]

    in_maps = _build_inmaps(x, W_dq, W_uq, W_qr, W_dkv, W_uk, W_uv, W_kr, W_o)

    res = run_bass_kernel_spmd(nc, in_maps, core_ids=list(range(8)))
    outs = [np.asarray(r["out"], dtype=np.float32) for r in res.results]
    out0 = outs[0] + outs[1] + outs[2] + outs[3]
    out1 = outs[4] + outs[5] + outs[6] + outs[7]
    return np.stack([out0, out1]).astype(np.float32)


# revision 32
# speedup vs baseline: 1.0144x; 1.0144x over previous
"""Multi-Head Latent Attention (MLA) Bass kernel for 8 trn2 NeuronCores.

Sharding: core c handles batch b=c//4 and head group hg=c%4 (4 of 16 heads).
Host folds the q-path compression (W_dq @ W_uq_hg, W_dq @ W_qr_hg) into one
[2048, 768] matrix per core so q projects straight from x — no c_q compute.
Everything on-device runs in bf16 (fp32 PSUM accumulation), which lets all
intermediates (kcT/krT/vS) stay SBUF-resident across the whole kernel: the
only DRAM traffic is x in, weights in, out rows out.

Per t-block g (512 tokens): project q/kv for the block, then run causal
attention for q-block g over key chunks 0..4g+3 (all SBUF-resident), then the
W_o partial for those rows. The softmax row-sum reciprocal is broadcast
across partitions with a K=1 outer-product matmul instead of a DMA roundtrip.

Host sums the 4 partial outputs per batch.
"""

import numpy as np

T = 2048
C = 2048
QC = 1536
KV = 512
NH = 16
DH = 128
R = 64
TB = 512           # T block / q-group width
NTB = T // TB      # 4
SCALE = 1.0 / float(np.sqrt(DH + R))
ROPE_BASE = 10000.0

_CACHE = {}


def _build_nc(repeat=1):
    import concourse.bacc as bacc
    import concourse.mybir as mybir
    import concourse.tile as tile

    DT = mybir.dt.bfloat16
    F32 = mybir.dt.float32
    F32R = mybir.dt.float32r

    nc = bacc.Bacc("TRN2", target_bir_lowering=False, debug=False)

    xT = nc.dram_tensor("xT", [C, T], DT, kind="ExternalInput")
    wq = nc.dram_tensor("wq", [C, 768], DT, kind="ExternalInput")
    wdkv = nc.dram_tensor("wdkv", [C, KV], DT, kind="ExternalInput")
    wuk = nc.dram_tensor("wuk", [KV, 512], DT, kind="ExternalInput")
    wuv = nc.dram_tensor("wuv", [KV, 512], DT, kind="ExternalInput")
    wkr = nc.dram_tensor("wkr", [KV, 256], DT, kind="ExternalInput")
    wo = nc.dram_tensor("wo", [512, C], DT, kind="ExternalInput")
    cosd = nc.dram_tensor("cosd", [128, T], F32, kind="ExternalInput")
    sind = nc.dram_tensor("sind", [128, T], F32, kind="ExternalInput")
    maskd = nc.dram_tensor("maskd", [TB, TB], DT, kind="ExternalInput")
    onesd = nc.dram_tensor("onesd", [128, 1], DT, kind="ExternalInput")
    rotmd = nc.dram_tensor("rotmd", [128, 128], DT, kind="ExternalInput")
    out = nc.dram_tensor("out", [T, C], DT, kind="ExternalOutput")

    with tile.TileContext(nc) as tc:
        for _rep in range(repeat):
            _emit_body(nc, tc, mybir,
                       xT, wq, wdkv, wuk, wuv, wkr, wo,
                       cosd, sind, maskd, onesd, rotmd, out)

    nc.compile()
    return nc


def _emit_body(nc, tc, mybir,
               xT, wq, wdkv, wuk, wuv, wkr, wo,
               cosd, sind, maskd, onesd, rotmd, out):
    DT = mybir.dt.bfloat16
    F32 = mybir.dt.float32
    F32R = mybir.dt.float32r
    AF = mybir.ActivationFunctionType

    with (
        tc.tile_pool(name="sb", bufs=1) as sp,
        tc.tile_pool(name="ps", bufs=1, space="PSUM") as pp,
    ):
        # resident weights + tables. wq streams per-k-chunk on the sync queue
        # so block 0's first matmuls start as soon as chunk 0 lands; the x
        # block streams in parallel on the scalar queue.
        wq_sb = [sp.tile([128, 768], DT, name=f"wq_sb{k}") for k in range(16)]
        for k in range(16):
            nc.sync.dma_start(wq_sb[k][:], wq[128 * k:128 * (k + 1), :])
        wdkv_sb = sp.tile([128, 16, KV], DT, name="wdkv_sb")
        wuk_sb = sp.tile([128, 4, 512], DT, name="wuk_sb")
        wuv_sb = sp.tile([128, 4, 512], DT, name="wuv_sb")
        wkr_sb = sp.tile([128, 4, 256], DT, name="wkr_sb")
        wo_sb = sp.tile([128, 4, C], DT, name="wo_sb")
        mask_sb = sp.tile([128, 4, TB], DT, name="mask_sb")
        ones_sb = sp.tile([128, 1], DT, name="ones_sb")
        rotm_sb = sp.tile([128, 128], DT, name="rotm_sb")

        def load_late_weights():
            nc.scalar.dma_start(wdkv_sb[:], wdkv.rearrange("(k p) n -> p k n", p=128))
            nc.sync.dma_start(wuk_sb[:], wuk.rearrange("(k p) n -> p k n", p=128))
            nc.scalar.dma_start(wuv_sb[:], wuv.rearrange("(k p) n -> p k n", p=128))
            nc.sync.dma_start(wkr_sb[:], wkr.rearrange("(k p) n -> p k n", p=128))
            nc.sync.dma_start(mask_sb[:], maskd.rearrange("(j p) n -> p j n", p=128))
            nc.sync.dma_start(ones_sb[:], onesd[:])
            nc.sync.dma_start(rotm_sb[:], rotmd[:])
            nc.scalar.dma_start(wo_sb[:], wo.rearrange("(h p) n -> p h n", p=128))

        # prewarm the Act function table so the first real exp/copy isn't
        # serialized behind LoadActFuncSet (its input is never consumed)
        dum = sp.tile([1, 16], F32, name="dum")
        nc.scalar.activation(dum[:], dum[:], AF.Exp)

        # resident intermediates (whole-T K/V for this head group)
        kcT = sp.tile([128, 4, T], DT, name="kcT")
        krT = sp.tile([128, 2, T], DT, name="krT")
        vS = sp.tile([128, 16, 512], DT, name="vS")

        def rope_begin(ps_t, cos_sb):
            # ps_t [128, TB]: rows [64 head 2p | 64 head 2p+1] rope dims.
            # Stage raw rope input to SBUF (Act) and take the cos term (DVE);
            # both read PSUM so the bank frees after these two ops.
            st = sp.tile([128, TB], DT, name="rst", tag="rst", bufs=3)
            nc.scalar.copy(st[:], ps_t[:])
            t1 = sp.tile([128, TB], F32, name="rp1", tag="rp1", bufs=3)
            nc.vector.tensor_mul(t1[:], ps_t[:], cos_sb[:])
            return st, t1

        def rope_finish(rb, dst, sin_sb):
            # rotate-half as a signed-permutation matmul on PE; emitted a
            # matmul-group later so the staging copy is done by the time the
            # PE reaches it (PE executes its queue in program order).
            st, t1 = rb
            ps2 = pp.tile([128, TB], F32, name="ps_r", tag="ps", bufs=3)
            nc.tensor.matmul(ps2[:], rotm_sb[:], st[:], start=True, stop=True)
            sh = sp.tile([128, TB], F32, name="rp2", tag="rp2", bufs=2)
            nc.vector.tensor_mul(sh[:], ps2[:], sin_sb[:])
            nc.vector.tensor_add(dst, t1[:], sh[:])

        for gp in range(2):
            pair = (2 * gp, 2 * gp + 1)
            # stage x/cos/sin for both blocks of the pair; per-k tiles keep
            # DMA-arrival dependencies exact.
            xb, cosv, sinv = {}, {}, {}
            for g in pair:
                tc0 = TB * g
                xb[g] = [sp.tile([128, TB], DT, name=f"xblk{g % 2}_{k}",
                                 tag=f"xblk{g % 2}_{k}", bufs=1)
                         for k in range(16)]
                for k in range(16):
                    eng = nc.scalar if (gp == 0 or k % 2 == 0) else nc.sync
                    eng.dma_start(xb[g][k][:],
                                  xT[128 * k:128 * (k + 1), tc0:tc0 + TB])
                cosv[g] = sp.tile([128, TB], F32, name="cos_sb",
                                  tag=f"cos{g % 2}", bufs=1)
                nc.scalar.dma_start(cosv[g][:], cosd[:, tc0:tc0 + TB])
                sinv[g] = sp.tile([128, TB], F32, name="sin_sb",
                                  tag=f"sin{g % 2}", bufs=1)
                nc.scalar.dma_start(sinv[g][:], sind[:, tc0:tc0 + TB])
            if gp == 0:
                load_late_weights()

            def ps_pair():
                # one PSUM accumulator per block of the pair, on separate
                # tags so their WAR chains stay independent
                return {pair[0]: pp.tile([128, TB], F32, name="ps",
                                         tag="ps", bufs=3),
                        pair[1]: pp.tile([128, TB], F32, name="ps_s",
                                         tag="ps_qk", bufs=3)}

            # ---- projections, k-inner over the pair: each weight slice is
            # stationary for 2 consecutive matmuls ----
            qc_t = {g: sp.tile([128, 4, TB], DT, name="qc_t",
                               tag=f"qc_t{g % 2}", bufs=1) for g in pair}
            qr_t = {g: sp.tile([128, 2, TB], DT, name="qr_t",
                               tag=f"qr_t{g % 2}", bufs=1) for g in pair}
            qr_rb = {}
            for m in range(6):
                pst = ps_pair()
                for k in range(16):
                    for g in pair:
                        nc.tensor.matmul(pst[g][:],
                                         wq_sb[k][:, 128 * m:128 * (m + 1)],
                                         xb[g][k][:],
                                         start=(k == 0), stop=(k == 15))
                for g in pair:
                    if m < 4:
                        nc.vector.tensor_copy(qc_t[g][:, m, :], pst[g][:])
                    else:
                        qr_rb[(g, m - 4)] = rope_begin(pst[g], cosv[g])
            ckv = {g: sp.tile([128, 4, TB], DT, name="ckv",
                              tag=f"ckv{g % 2}", bufs=1) for g in pair}
            for m in range(4):
                pst = ps_pair()
                for k in range(16):
                    for g in pair:
                        nc.tensor.matmul(pst[g][:],
                                         wdkv_sb[:, k, 128 * m:128 * (m + 1)],
                                         xb[g][k][:],
                                         start=(k == 0), stop=(k == 15))
                for g in pair:
                    nc.vector.tensor_copy(ckv[g][:, m, :], pst[g][:])
                if m < 2:
                    # both blocks' rot matmuls back-to-back: they share the
                    # stationary rotation matrix
                    for g in pair:
                        rope_finish(qr_rb[(g, m)], qr_t[g][:, m, :], sinv[g])
            for h in range(4):
                pst = ps_pair()
                for k in range(4):
                    for g in pair:
                        nc.tensor.matmul(pst[g][:],
                                         wuk_sb[:, k, 128 * h:128 * (h + 1)],
                                         ckv[g][:, k, :],
                                         start=(k == 0), stop=(k == 3))
                for g in pair:
                    nc.vector.tensor_copy(kcT[:, h, TB * g:TB * (g + 1)],
                                          pst[g][:])
            kr_rb = {}
            for p in range(2):
                pst = ps_pair()
                for k in range(4):
                    for g in pair:
                        nc.tensor.matmul(pst[g][:],
                                         wkr_sb[:, k, 128 * p:128 * (p + 1)],
                                         ckv[g][:, k, :],
                                         start=(k == 0), stop=(k == 3))
                for g in pair:
                    kr_rb[(g, p)] = rope_begin(pst[g], cosv[g])
            for tkc in range(4):
                pst = ps_pair()
                for k in range(4):
                    for g in pair:
                        nc.tensor.matmul(pst[g][:],
                                         ckv[g][:, k, 128 * tkc:128 * (tkc + 1)],
                                         wuv_sb[:, k, :],
                                         start=(k == 0), stop=(k == 3))
                for g in pair:
                    nc.scalar.copy(vS[:, 4 * g + tkc, :], pst[g][:])
                if 1 <= tkc <= 2:
                    for g in pair:
                        rope_finish(kr_rb[(g, tkc - 1)],
                                    krT[:, tkc - 1, TB * g:TB * (g + 1)],
                                    sinv[g])

            # ---- attention + W_o per block of the pair ----
            for g in pair:
                tc0 = TB * g
                nch = 4 * (g + 1)
                avn = sp.tile([128, 4, TB], DT, name="avn", tag="avn", bufs=2)
                for h in range(4):
                    hp, r0 = h // 2, 64 * (h % 2)
                    ps_av = pp.tile([128, TB], F32, name="ps_av", tag="ps_av",
                                    bufs=1)
                    ps_sum = pp.tile([1, TB], F32, name="ps_sum", tag="ps_sum",
                                     bufs=1)

                    def qstart(c):
                        # columns tq < tk are fully masked; chunk 0 stays full
                        # (PSUM base write must cover all columns)
                        if c <= 4 * g or c == 0:
                            return 0
                        return 128 * (c - 4 * g)

                    def qk(c):
                        s = qstart(c)
                        ps_s = pp.tile([128, TB], F32, name="ps_s",
                                       tag="ps_qk", bufs=3)
                        nc.tensor.matmul(ps_s[:, s:],
                                         kcT[:, h, 128 * c:128 * (c + 1)],
                                         qc_t[g][:, h, s:],
                                         start=True, stop=False)
                        nc.tensor.matmul(ps_s[:, s:],
                                         krT[r0:r0 + 64, hp, 128 * c:128 * (c + 1)],
                                         qr_t[g][r0:r0 + 64, hp, s:],
                                         start=False, stop=True)
                        return ps_s

                    qkq = [qk(c) for c in range(min(3, nch))]
                    for c in range(nch):
                        cur = qkq.pop(0)
                        if c + 3 < nch:
                            qkq.append(qk(c + 3))
                        s = qstart(c)
                        ex = sp.tile([128, TB], DT, name="ex", tag="ex", bufs=4)
                        if c >= 4 * g:
                            er = sp.tile([128, TB], DT, name="er", tag="er",
                                         bufs=2)
                            nc.scalar.activation(er[:, s:], cur[:, s:], AF.Exp,
                                                 scale=SCALE)
                            nc.vector.tensor_mul(ex[:, s:], er[:, s:],
                                                 mask_sb[:, c - 4 * g, s:])
                        else:
                            nc.scalar.activation(ex[:, s:], cur[:, s:], AF.Exp,
                                                 scale=SCALE)
                        nc.tensor.matmul(ps_sum[:, s:], ones_sb[:], ex[:, s:],
                                         start=(c == 0), stop=(c == nch - 1))
                        nc.tensor.matmul(ps_av[:, s:],
                                         vS[:, c, 128 * h:128 * (h + 1)],
                                         ex[:, s:], start=(c == 0),
                                         stop=(c == nch - 1))
                    recip = sp.tile([1, TB], F32, name="recip", tag="recip",
                                    bufs=2)
                    nc.vector.reciprocal(recip[:], ps_sum[:])
                    bc = sp.tile([128, TB], F32, name="bc", tag="bc", bufs=1)
                    nc.gpsimd.partition_broadcast(bc[:], recip[:])
                    nc.vector.tensor_mul(avn[:, h, :], ps_av[:], bc[:])

                for tqc in range(4):
                    # h-outer/n-inner: 4 consecutive matmuls share the
                    # stationary avn chunk
                    pso = [pp.tile([128, 512], F32, name="ps_o", tag="ps_qk",
                                   bufs=3) for _ in range(2)]
                    pso += [pp.tile([128, 512], F32, name="ps", tag="ps",
                                    bufs=3) for _ in range(2)]
                    for h in range(4):
                        for n in range(4):
                            nc.tensor.matmul(
                                pso[n][:], avn[:, h, 128 * tqc:128 * (tqc + 1)],
                                wo_sb[:, h, 512 * n:512 * (n + 1)],
                                start=(h == 0), stop=(h == 3))
                    for n in range(4):
                        ost = sp.tile([128, 512], DT, name="ost", tag="ost",
                                      bufs=4)
                        if g == NTB - 1 or (tqc + n) % 2 == 0:
                            nc.vector.tensor_copy(ost[:], pso[n][:])
                        else:
                            nc.scalar.copy(ost[:], pso[n][:])
                        oeng = nc.sync if (tqc + n) % 2 == 0 else nc.scalar
                        oeng.dma_start(
                            out[tc0 + 128 * tqc:tc0 + 128 * (tqc + 1),
                                512 * n:512 * (n + 1)], ost[:])


def _rope_tables():
    inv = 1.0 / (ROPE_BASE ** (np.arange(0, R, 2, dtype=np.float32) / R))
    freqs = np.arange(T, dtype=np.float32)[:, None] * inv[None, :]       # [T, 32]
    emb = np.concatenate([freqs, freqs], axis=-1)                         # [T, 64]
    cosT = np.ascontiguousarray(np.cos(emb).T.astype(np.float32))         # [64, T]
    sinT = np.ascontiguousarray(np.sin(emb).T.astype(np.float32))
    cosd = np.concatenate([cosT, cosT], axis=0)                           # [128, T]
    sind = np.concatenate([sinT, sinT], axis=0)                           # [128, T]
    return cosd, sind


def _build_inmaps(x, W_dq, W_uq, W_qr, W_dkv, W_uk, W_uv, W_kr, W_o):
    import ml_dtypes
    BF16 = ml_dtypes.bfloat16

    cosd, sind = _rope_tables()
    maskv = (np.arange(TB)[:, None] <= np.arange(TB)[None, :]).astype(BF16)
    onesv = np.ones((128, 1), dtype=BF16)
    # rot(x)[i] = -x[i+32] (i<32), +x[i-32] (32<=i<64), per 64-row head block
    P64 = np.zeros((64, 64), dtype=np.float32)
    for i in range(32):
        P64[i, i + 32] = -1.0
        P64[i + 32, i] = 1.0
    Pm = np.zeros((128, 128), dtype=np.float32)
    Pm[0:64, 0:64] = P64
    Pm[64:128, 64:128] = P64
    rotmv = np.ascontiguousarray(Pm.T).astype(BF16)   # lhsT for out = Pm @ x

    # fold the q compression once for all heads, slice per core
    Wq_c = (W_dq @ W_uq).astype(np.float32)   # [2048, 2048]
    Wq_r = (W_dq @ W_qr).astype(np.float32)   # [2048, 1024]

    in_maps = []
    for core in range(8):
        b, hg = core // 4, core % 4
        wq_np = np.concatenate(
            [Wq_c[:, 512 * hg:512 * (hg + 1)], Wq_r[:, 256 * hg:256 * (hg + 1)]],
            axis=1).astype(BF16)
        in_maps.append({
            "xT": np.ascontiguousarray(x[b].T).astype(BF16),
            "wq": wq_np,
            "wdkv": W_dkv.astype(BF16),
            "wuk": np.ascontiguousarray(W_uk[:, 512 * hg:512 * (hg + 1)]).astype(BF16),
            "wuv": np.ascontiguousarray(W_uv[:, 512 * hg:512 * (hg + 1)]).astype(BF16),
            "wkr": np.ascontiguousarray(W_kr[:, 256 * hg:256 * (hg + 1)]).astype(BF16),
            "wo": np.ascontiguousarray(W_o[512 * hg:512 * (hg + 1), :]).astype(BF16),
            "cosd": cosd,
            "sind": sind,
            "maskd": maskv,
            "onesd": onesv,
            "rotmd": rotmv,
        })
    return in_maps


def kernel(**inputs):
    from concourse.bass_utils import run_bass_kernel_spmd

    x = np.asarray(inputs["x"], dtype=np.float32)
    W_dq = np.asarray(inputs["W_dq"], dtype=np.float32)
    W_uq = np.asarray(inputs["W_uq"], dtype=np.float32)
    W_qr = np.asarray(inputs["W_qr"], dtype=np.float32)
    W_dkv = np.asarray(inputs["W_dkv"], dtype=np.float32)
    W_uk = np.asarray(inputs["W_uk"], dtype=np.float32)
    W_uv = np.asarray(inputs["W_uv"], dtype=np.float32)
    W_kr = np.asarray(inputs["W_kr"], dtype=np.float32)
    W_o = np.asarray(inputs["W_o"], dtype=np.float32)

    if "nc" not in _CACHE:
        _CACHE["nc"] = _build_nc()
    nc = _CACHE["nc"]

    in_maps = _build_inmaps(x, W_dq, W_uq, W_qr, W_dkv, W_uk, W_uv, W_kr, W_o)

    res = run_bass_kernel_spmd(nc, in_maps, core_ids=list(range(8)))
    outs = [np.asarray(r["out"], dtype=np.float32) for r in res.results]
    out0 = outs[0] + outs[1] + outs[2] + outs[3]
    out1 = outs[4] + outs[5] + outs[6] + outs[7]
    return np.stack([out0, out1]).astype(np.float32)
